# revision 26
# baseline (speedup 1.0000x reference)
"""Trainium2 Bass kernel for nn_MetricModel (retrieval_knn).

Computation (per sample b):
  protos[c,d] = mean of x[d,:] over pixels with label c+1   (C=16, D=128)
  dist[c,n]   = sum_d |x[d,n] - protos[c,d]|                (L1 metric)
  pd0 = exp(-dist);  ce[n] = logsumexp_c(pd0) - pd0[label]  (min-shift of the
        reference cancels exactly; values collapse identically in f32)
  loss = sum_b masked_mean(ce)

Sharding: 8 cores = (batch b, spatial half h).  Each core receives the FULL
sample's x/labels (reordered so its own half comes first; prototype sums are
pixel-order invariant) and computes distances + CE only for its local 8192
pixels.  Host combines per-core masked sums/counts into the final scalar.

Per-core pipeline (~104 us modeled by TimelineSim, 94 us CoreSim span):
  - x arrives bf16 from host [128(d), 16384(n)]; local half staged in SBUF,
    pixel-major xT tiles made by DMA-xbar transposes (non-local half
    transposed straight from DRAM); onehot tiles built on idle GPSIMD
  - protosum[16,128] = 128 accumulating PE matmuls (onehot.T @ xT);
    counts via DVE reduce + tiny PE matmuls; protos = protosum/counts
  - L1 distance via |v| = 2*relu(v) - v (the ISA has no fp abs ALU op):
    relu(x - protos[:,c]) in ONE fused DVE tensor_scalar (subtract, max 0)
    at 4x bf16 rate; PE matmuls with indicator weights E_c reduce over d,
    and a -0.5*ones matmul folds the class-independent -sum_d(x)/2 term
    into the same PSUM accumulation; + sum_d protos via ACT bias on the
    PSUM->SBUF copy (scale=2) -> dist = 2*R + ps1 - xs1 exactly
  - dist chunks [16,512] transposed to pixel-major by DMA-xbar; softmax-CE
    on [128, 16, 16] quarters (ACT exp/ln + DVE reduces), masked accumulation
  - out [128,2] = per-partition masked CE sums and mask counts; host combines
  - dbg [128,4,16] = raw distances of 512 pixels (validates the pipeline on
    HW, since exp(-dist~100) underflows and the scalar alone cannot)
"""

import numpy as np
from contextlib import ExitStack

import concourse.bass as bass
import concourse.mybir as mybir
import concourse.bacc as bacc
import concourse.tile as tile
from concourse.bass_utils import run_bass_kernel_spmd

B, D, H, W, C = 4, 128, 128, 128, 16
N = H * W          # 16384 pixels per sample
NLOC = N // 2      # 8192 pixels per core
NT = N // 128      # 128 pixel-tiles of 128
GLOC = NLOC // 128  # 64 local tiles
NCHUNK = NLOC // 512  # 16 dist chunks of 512 pixels

FP32 = mybir.dt.float32
BF16 = mybir.dt.bfloat16
I32 = mybir.dt.int32
AF = mybir.ActivationFunctionType
ALU = mybir.AluOpType
AX = mybir.AxisListType


def build_body(tc, out_ap, dbg_ap, xbin_ap, lab_ap, msk_ap):
    nc = tc.nc
    with ExitStack() as ctx:
        pc = ctx.enter_context(tc.tile_pool(name="consts", bufs=1))
        pxb = ctx.enter_context(tc.tile_pool(name="xb", bufs=1))
        pxT = ctx.enter_context(tc.tile_pool(name="xT", bufs=1))
        poh = ctx.enter_context(tc.tile_pool(name="oh", bufs=1))
        plab = ctx.enter_context(tc.tile_pool(name="lab", bufs=1))
        pad = ctx.enter_context(tc.tile_pool(name="ad", bufs=8))
        pdsb = ctx.enter_context(tc.tile_pool(name="dsb", bufs=4))
        ppost = ctx.enter_context(tc.tile_pool(name="post", bufs=1))
        psm = ctx.enter_context(tc.tile_pool(name="psm", bufs=1, space="PSUM"))
        psproto = ctx.enter_context(tc.tile_pool(name="psproto", bufs=1, space="PSUM"))
        psdist = ctx.enter_context(tc.tile_pool(name="psdist", bufs=6, space="PSUM"))

        # ---------------- constants ----------------
        iotaC_i = pc.tile([128, C], I32)
        nc.gpsimd.iota(iotaC_i[:], pattern=[[1, C]], base=1, channel_multiplier=0)
        iotaCf = pc.tile([128, C], FP32)
        nc.vector.tensor_copy(iotaCf[:], iotaC_i[:])

        idn_i = pc.tile([128, 128], I32)  # j - p
        nc.gpsimd.iota(idn_i[:], pattern=[[1, 128]], base=0, channel_multiplier=-1)
        I128f = pc.tile([128, 128], FP32)
        nc.vector.tensor_scalar(I128f[:], idn_i[:], 0, None, op0=ALU.is_equal)
        I16b = pc.tile([16, 16], BF16)
        nc.vector.tensor_copy(I16b[:], I128f[0:16, 0:16])

        E_i = pc.tile([128, C, C], I32)  # c - m
        nc.gpsimd.iota(E_i[:], pattern=[[1, C], [-1, C]], base=0, channel_multiplier=0)
        E_b = pc.tile([128, C, C], BF16)
        nc.vector.tensor_scalar(E_b[:], E_i[:], 0, None, op0=ALU.is_equal)

        ones128f = pc.tile([128, 1], FP32)
        nc.vector.memset(ones128f[:], 1.0)
        negh = pc.tile([128, C], BF16)  # -0.5 everywhere: adds -xs1/2 to psum
        nc.vector.memset(negh[:], -0.5)

        # ---------------- inputs ----------------
        labi = plab.tile([128, 128], I32)
        nc.sync.dma_start(labi[:], lab_ap[:])
        mski = plab.tile([64, 128], I32)
        nc.sync.dma_start(mski[:], msk_ap[:])

        # labels/mask to f32, transposed (labT[p, t] = label[t*128+p])
        labf = plab.tile([128, 128], FP32)
        nc.vector.tensor_copy(labf[:], labi[:])
        ps1 = psm.tile([128, 128], FP32, tag="ps_misc")
        nc.tensor.transpose(ps1[:], labf[:], I128f[:])
        labT = plab.tile([128, 128], FP32)
        nc.vector.tensor_copy(labT[:], ps1[:])

        mskf = plab.tile([64, 128], FP32)
        nc.vector.tensor_copy(mskf[:], mski[:])
        ps2 = psm.tile([128, 128], FP32, tag="ps_misc")
        nc.tensor.transpose(ps2[0:128, 0:64], mskf[:], I128f[0:64, 0:64])
        mskT = plab.tile([128, GLOC], FP32)
        nc.vector.tensor_copy(mskT[:], ps2[0:128, 0:64])

        # ------- x arrives as bf16 from host; DMA in chunks, transpose each -------
        # xT out[p, e, d] = xb[d, 128*e + p] (e-major): tile t = pixels [128t,...)
        xb = pxb.tile([128, NLOC], BF16)  # local half only
        xT = pxT.tile([128, NT, 128], BF16)   # [pix_in_tile, tile, d]
        oh = poh.tile([128, NT, C], BF16)     # onehot per tile (bf16 weights)
        for t in range(NT):
            nc.gpsimd.tensor_scalar(
                oh[:, t, :], iotaCf[:], labT[:, t : t + 1], None, op0=ALU.is_equal
            )
        NTR = 8
        TRW = N // NTR  # 2048 cols per transpose
        for b in range(NTR):
            if b < NTR // 2:
                # local half: stage in SBUF (also feeds distances), transpose from SBUF
                nc.sync.dma_start(xb[:, bass.ts(b, TRW)], xbin_ap[:, bass.ts(b, TRW)])
                nc.sync.dma_start_transpose(
                    xT[:, bass.ts(b, NT // NTR), :], xb[:, bass.ts(b, TRW)]
                )
            else:
                # non-local half: only protos need it -> transpose straight from DRAM
                nc.sync.dma_start_transpose(
                    xT[:, bass.ts(b, NT // NTR), :], xbin_ap[:, bass.ts(b, TRW)]
                )

        # ---------------- prototypes ----------------
        protosum = psproto.tile([16, 128], FP32, tag="protosum")
        for t in range(NT):
            nc.tensor.matmul(
                protosum[:], oh[:, t, :], xT[:, t, :],
                start=(t == 0), stop=(t == NT - 1),
            )
        # counts: reduce onehot over tiles, then PE partition-sum
        counts128 = plab.tile([128, C], FP32)
        oh_v = oh[:].rearrange("p t c -> p c t")
        nc.vector.tensor_reduce(counts128[:], oh_v, axis=AX.X, op=ALU.add)
        cnt_ps = psm.tile([1, C], FP32, tag="ps_misc")
        nc.tensor.matmul(cnt_ps[:], ones128f[:], counts128[:], start=True, stop=True)
        crow = plab.tile([1, C], FP32)
        nc.vector.tensor_copy(crow[:], cnt_ps[:])
        cntT_ps = psm.tile([16, 1], FP32, tag="ps_misc")
        nc.tensor.transpose(cntT_ps[:], crow[:], I128f[0:1, 0:1])
        counts = plab.tile([16, 1], FP32)
        nc.vector.tensor_copy(counts[:], cntT_ps[:])
        recip = plab.tile([16, 1], FP32)
        nc.vector.reciprocal(recip[:], counts[:])
        protom = plab.tile([16, 128], FP32)
        nc.vector.tensor_scalar(protom[:], protosum[:], recip[:], None, op0=ALU.mult)
        pT_ps = psm.tile([128, 128], FP32, tag="ps_misc")
        nc.tensor.transpose(pT_ps[0:128, 0:16], protom[:], I128f[0:16, 0:16])
        protosTf = plab.tile([128, C], FP32)
        nc.vector.tensor_copy(protosTf[:], pT_ps[0:128, 0:16])

        # |v| = 2*relu(v) - v  =>  dist[c,n] = 2*(sum_d relu(x-p) - xs1[n]/2) + ps1[c]
        # the -xs1/2 term lands in the same PSUM via a -0.5*ones matmul.
        ps1s = plab.tile([16, 1], FP32)
        nc.vector.tensor_reduce(ps1s[:], protom[:], axis=AX.X, op=ALU.add)

        # ---------------- distances ----------------
        dists_q = [
            ppost.tile([128, GLOC // 4, C], BF16, name=f"dists{q}", tag=f"dists{q}")
            for q in range(4)
        ]
        BLKW = 2048
        NBLK = NLOC // BLKW  # 4 blocks x 4 chunks of 512
        for blk in range(NBLK):
            dist_ps = [psdist.tile([16, 512], FP32, tag="dist", name=f"dist{blk}_{j}") for j in range(4)]
            for c in range(C):
                ad = pad.tile([128, BLKW], BF16, tag="ad")
                nc.vector.tensor_scalar(
                    ad[:], xb[:, bass.ds(BLKW * blk, BLKW)],
                    protosTf[:, c : c + 1], 0.0,
                    op0=ALU.subtract, op1=ALU.max,
                )
                for j in range(4):
                    nc.tensor.matmul(
                        dist_ps[j][:], E_b[:, c, :], ad[:, bass.ts(j, 512)],
                        start=(c == 0), stop=False, skip_group_check=True,
                    )
            for j in range(4):
                nc.tensor.matmul(
                    dist_ps[j][:], negh[:],
                    xb[:, bass.ds(BLKW * blk + 512 * j, 512)],
                    start=False, stop=True, skip_group_check=True,
                )
            for j in range(4):
                # dsb = 2*psum + ps1 (fused into the PSUM->SBUF copy), bf16
                dsb = pdsb.tile([16, 512], BF16)
                nc.scalar.activation(
                    dsb[:], dist_ps[j][:], AF.Identity, bias=ps1s[:], scale=2.0
                )
                # DMA-xbar transpose [16,512] -> [128, 4, 16] (pixel-major)
                nc.sync.dma_start_transpose(
                    dists_q[blk][:, 4 * j : 4 * j + 4, :], dsb[:]
                )

        # ---------------- post-processing (softmax CE) ----------------
        ohp = poh.tile([128, GLOC, C], BF16)  # local-half onehot
        for g in range(GLOC):
            nc.gpsimd.tensor_scalar(
                ohp[:, g, :], iotaCf[:], labT[:, g : g + 1], None, op0=ALU.is_equal
            )
        # per-quarter post (overlaps with the next distance block's matmuls)
        GB = GLOC // NBLK  # 16 tiles per quarter
        mce = ppost.tile([128, GLOC], FP32)
        for q in range(NBLK):
            g0 = q * GB
            e = ppost.tile([128, GB, C], FP32, name=f"e{q}", tag="e", bufs=2)
            z = ppost.tile([128, GB, C], FP32, name=f"z{q}", tag="z", bufs=2)
            s = ppost.tile([128, GB], FP32, name=f"s{q}", tag="s", bufs=2)
            lse = ppost.tile([128, GB], FP32, name=f"lse{q}", tag="lse", bufs=2)
            dsel = ppost.tile([128, GB], FP32, name=f"dsel{q}", tag="dsel", bufs=2)
            esel = ppost.tile([128, GB], FP32, name=f"esel{q}", tag="esel", bufs=2)
            ce = ppost.tile([128, GB], FP32, name=f"ce{q}", tag="ce", bufs=2)
            e2 = e[:].rearrange("p g c -> p (g c)")
            z2 = z[:].rearrange("p g c -> p (g c)")
            dq = dists_q[q][:].rearrange("p g c -> p (g c)")
            nc.scalar.activation(e2, dq, AF.Exp, scale=-1.0)
            nc.scalar.activation(z2, e2, AF.Exp)
            nc.vector.tensor_reduce(s[:], z[:], axis=AX.X, op=ALU.add)
            nc.scalar.activation(lse[:], s[:], AF.Ln)
            # dist at label class: sum_c dists*onehot, then exp(-x)
            oq = ohp[:, g0 : g0 + GB, :].rearrange("p g c -> p (g c)")
            nc.vector.tensor_mul(z2, dq, oq)
            nc.vector.tensor_reduce(dsel[:], z[:], axis=AX.X, op=ALU.add)
            nc.scalar.activation(esel[:], dsel[:], AF.Exp, scale=-1.0)
            nc.vector.tensor_sub(ce[:], lse[:], esel[:])
            nc.vector.tensor_mul(mce[:, g0 : g0 + GB], ce[:], mskT[:, g0 : g0 + GB])

        dbg = ppost.tile([128, 4, C], FP32)
        nc.vector.tensor_copy(dbg[:], dists_q[0][:, 0:4, :])
        nc.sync.dma_start(dbg_ap[:], dbg[:])

        outsb = ppost.tile([128, 2], FP32)
        nc.vector.tensor_reduce(outsb[:, 0:1], mce[:], axis=AX.X, op=ALU.add)
        nc.vector.tensor_reduce(outsb[:, 1:2], mskT[:], axis=AX.X, op=ALU.add)
        nc.sync.dma_start(out_ap[:], outsb[:])


_CACHE = {}


def _build():
    if "nc" in _CACHE:
        return _CACHE["nc"]
    nc = bacc.Bacc("TRN2", target_bir_lowering=False, debug=False, num_devices=8)
    xb_ap = nc.dram_tensor("xb", [128, N], BF16, kind="ExternalInput").ap()
    lab_ap = nc.dram_tensor("lab", [128, 128], I32, kind="ExternalInput").ap()
    msk_ap = nc.dram_tensor("msk", [64, 128], I32, kind="ExternalInput").ap()
    out_ap = nc.dram_tensor("acc", [128, 2], FP32, kind="ExternalOutput").ap()
    dbg_ap = nc.dram_tensor("dbg", [128, 4, C], FP32, kind="ExternalOutput").ap()
    with tile.TileContext(nc) as tc:
        build_body(tc, out_ap, dbg_ap, xb_ap, lab_ap, msk_ap)
    nc.compile()
    _CACHE["nc"] = nc
    return nc


def make_in_maps(batch_x, batch_label, batch_mask):
    import ml_dtypes

    xb_all = np.asarray(batch_x).astype(ml_dtypes.bfloat16)  # one 32->16MB cast
    in_maps = []
    for k in range(8):
        b, h = k // 2, k % 2
        loc = slice(64 * h, 64 * h + 64)
        oth = slice(64 - 64 * h, 128 - 64 * h)
        xf = np.concatenate(
            [
                xb_all[b][:, loc, :].reshape(128, NLOC),
                xb_all[b][:, oth, :].reshape(128, NLOC),
            ],
            axis=1,
        )
        lab = np.concatenate([batch_label[b][loc], batch_label[b][oth]], axis=0)
        msk = batch_mask[b, 0][loc]
        in_maps.append(
            {
                "xb": xf,
                "lab": np.ascontiguousarray(lab, dtype=np.int32),
                "msk": np.ascontiguousarray(msk, dtype=np.int32),
            }
        )
    return in_maps


def combine(outs):
    total = 0.0
    for b in range(4):
        s = float(outs[2 * b][:, 0].sum()) + float(outs[2 * b + 1][:, 0].sum())
        m = float(outs[2 * b][:, 1].sum()) + float(outs[2 * b + 1][:, 1].sum())
        total += np.float32(s) / np.float32(m)
    return np.asarray(total, dtype=np.float32)


def kernel(batch_x, batch_label, batch_mask, **kw):
    nc = _build()
    in_maps = make_in_maps(
        np.asarray(batch_x), np.asarray(batch_label), np.asarray(batch_mask)
    )
    res = run_bass_kernel_spmd(nc, in_maps, core_ids=list(range(8)), **kw)
    outs = [r["acc"] for r in res.results]
    out = combine(outs)
    if kw:
        return out, res
    return out


# revision 27
# speedup vs baseline: 1.0241x; 1.0241x over previous
"""Trainium2 Bass kernel for nn_MetricModel (retrieval_knn).

Computation (per sample b):
  protos[c,d] = mean of x[d,:] over pixels with label c+1   (C=16, D=128)
  dist[c,n]   = sum_d |x[d,n] - protos[c,d]|                (L1 metric)
  pd0 = exp(-dist);  ce[n] = logsumexp_c(pd0) - pd0[label]  (min-shift of the
        reference cancels exactly; values collapse identically in f32)
  loss = sum_b masked_mean(ce)

Sharding: 8 cores = (batch b, spatial half h).  Each core receives the FULL
sample's x/labels (reordered so its own half comes first; prototype sums are
pixel-order invariant) and computes distances + CE only for its local 8192
pixels.  Host combines per-core masked sums/counts into the final scalar.

Per-core pipeline (~104 us modeled by TimelineSim, 94 us CoreSim span):
  - x arrives bf16 from host [128(d), 16384(n)]; local half staged in SBUF,
    pixel-major xT tiles made by DMA-xbar transposes (non-local half
    transposed straight from DRAM); onehot tiles built on idle GPSIMD
  - protosum[16,128] = 128 accumulating PE matmuls (onehot.T @ xT);
    counts via DVE reduce + tiny PE matmuls; protos = protosum/counts
  - L1 distance via |v| = 2*relu(v) - v (the ISA has no fp abs ALU op):
    relu(x - protos[:,c]) in ONE fused DVE tensor_scalar (subtract, max 0)
    at 4x bf16 rate; PE matmuls with indicator weights E_c reduce over d,
    and a -0.5*ones matmul folds the class-independent -sum_d(x)/2 term
    into the same PSUM accumulation; + sum_d protos via ACT bias on the
    PSUM->SBUF copy (scale=2) -> dist = 2*R + ps1 - xs1 exactly
  - dist chunks [16,512] transposed to pixel-major by DMA-xbar; softmax-CE
    on [128, 16, 16] quarters (ACT exp/ln + DVE reduces), masked accumulation
  - out [128,2] = per-partition masked CE sums and mask counts; host combines
  - dbg [128,4,16] = raw distances of 512 pixels (validates the pipeline on
    HW, since exp(-dist~100) underflows and the scalar alone cannot)
"""

import numpy as np
from contextlib import ExitStack

import concourse.bass as bass
import concourse.mybir as mybir
import concourse.bacc as bacc
import concourse.tile as tile
from concourse.bass_utils import run_bass_kernel_spmd

B, D, H, W, C = 4, 128, 128, 128, 16
N = H * W          # 16384 pixels per sample
NLOC = N // 2      # 8192 pixels per core
NT = N // 128      # 128 pixel-tiles of 128
GLOC = NLOC // 128  # 64 local tiles
NCHUNK = NLOC // 512  # 16 dist chunks of 512 pixels

FP32 = mybir.dt.float32
BF16 = mybir.dt.bfloat16
I32 = mybir.dt.int32
AF = mybir.ActivationFunctionType
ALU = mybir.AluOpType
AX = mybir.AxisListType


def build_body(tc, out_ap, dbg_ap, xbin_ap, lab_ap, msk_ap):
    nc = tc.nc
    with ExitStack() as ctx:
        pc = ctx.enter_context(tc.tile_pool(name="consts", bufs=1))
        pxb = ctx.enter_context(tc.tile_pool(name="xb", bufs=1))
        pxT = ctx.enter_context(tc.tile_pool(name="xT", bufs=1))
        poh = ctx.enter_context(tc.tile_pool(name="oh", bufs=1))
        plab = ctx.enter_context(tc.tile_pool(name="lab", bufs=1))
        pad = ctx.enter_context(tc.tile_pool(name="ad", bufs=8))
        pdsb = ctx.enter_context(tc.tile_pool(name="dsb", bufs=4))
        ppost = ctx.enter_context(tc.tile_pool(name="post", bufs=1))
        psm = ctx.enter_context(tc.tile_pool(name="psm", bufs=1, space="PSUM"))
        psproto = ctx.enter_context(tc.tile_pool(name="psproto", bufs=1, space="PSUM"))
        psdist = ctx.enter_context(tc.tile_pool(name="psdist", bufs=6, space="PSUM"))

        # ---------------- constants ----------------
        iotaC_i = pc.tile([128, C], I32)
        nc.gpsimd.iota(iotaC_i[:], pattern=[[1, C]], base=1, channel_multiplier=0)
        iotaCf = pc.tile([128, C], FP32)
        nc.vector.tensor_copy(iotaCf[:], iotaC_i[:])

        idn_i = pc.tile([128, 128], I32)  # j - p
        nc.gpsimd.iota(idn_i[:], pattern=[[1, 128]], base=0, channel_multiplier=-1)
        I128f = pc.tile([128, 128], FP32)
        nc.vector.tensor_scalar(I128f[:], idn_i[:], 0, None, op0=ALU.is_equal)
        I16b = pc.tile([16, 16], BF16)
        nc.vector.tensor_copy(I16b[:], I128f[0:16, 0:16])

        E_i = pc.tile([128, C, C], I32)  # c - m
        nc.gpsimd.iota(E_i[:], pattern=[[1, C], [-1, C]], base=0, channel_multiplier=0)
        E_b = pc.tile([128, C, C], BF16)
        nc.vector.tensor_scalar(E_b[:], E_i[:], 0, None, op0=ALU.is_equal)

        ones128f = pc.tile([128, 1], FP32)
        nc.vector.memset(ones128f[:], 1.0)
        negh = pc.tile([128, C], BF16)  # -0.5 everywhere: adds -xs1/2 to psum
        nc.vector.memset(negh[:], -0.5)

        # ---------------- inputs ----------------
        labi = plab.tile([128, 128], I32)
        nc.sync.dma_start(labi[:], lab_ap[:])
        mski = plab.tile([64, 128], I32)
        nc.sync.dma_start(mski[:], msk_ap[:])

        # labels/mask to f32, transposed (labT[p, t] = label[t*128+p])
        labf = plab.tile([128, 128], FP32)
        nc.vector.tensor_copy(labf[:], labi[:])
        ps1 = psm.tile([128, 128], FP32, tag="ps_misc")
        nc.tensor.transpose(ps1[:], labf[:], I128f[:])
        labT = plab.tile([128, 128], FP32)
        nc.vector.tensor_copy(labT[:], ps1[:])

        mskf = plab.tile([64, 128], FP32)
        nc.vector.tensor_copy(mskf[:], mski[:])
        ps2 = psm.tile([128, 128], FP32, tag="ps_misc")
        nc.tensor.transpose(ps2[0:128, 0:64], mskf[:], I128f[0:64, 0:64])
        mskT = plab.tile([128, GLOC], FP32)
        nc.vector.tensor_copy(mskT[:], ps2[0:128, 0:64])

        # ------- x arrives as bf16 from host; DMA in chunks, transpose each -------
        # xT out[p, e, d] = xb[d, 128*e + p] (e-major): tile t = pixels [128t,...)
        xb = pxb.tile([128, NLOC], BF16)  # local half only
        xT = pxT.tile([128, NT, 128], BF16)   # [pix_in_tile, tile, d]
        oh = poh.tile([128, NT, C], BF16)     # onehot per tile (bf16 weights)
        for t in range(NT):
            nc.gpsimd.tensor_scalar(
                oh[:, t, :], iotaCf[:], labT[:, t : t + 1], None, op0=ALU.is_equal
            )
        NTR = 8
        TRW = N // NTR  # 2048 cols per transpose
        # all transposes straight from DRAM (protos critical path first) ...
        for b in range(NTR):
            nc.sync.dma_start_transpose(
                xT[:, bass.ts(b, NT // NTR), :], xbin_ap[:, bass.ts(b, TRW)]
            )
        # ... then stage the local half for the distance phase (needed later)
        for b in range(NTR // 2):
            nc.sync.dma_start(xb[:, bass.ts(b, TRW)], xbin_ap[:, bass.ts(b, TRW)])

        # ---------------- prototypes ----------------
        protosum = psproto.tile([16, 128], FP32, tag="protosum")
        for t in range(NT):
            nc.tensor.matmul(
                protosum[:], oh[:, t, :], xT[:, t, :],
                start=(t == 0), stop=(t == NT - 1),
            )
        # counts: reduce onehot over tiles, then PE partition-sum
        counts128 = plab.tile([128, C], FP32)
        oh_v = oh[:].rearrange("p t c -> p c t")
        nc.vector.tensor_reduce(counts128[:], oh_v, axis=AX.X, op=ALU.add)
        cnt_ps = psm.tile([1, C], FP32, tag="ps_misc")
        nc.tensor.matmul(cnt_ps[:], ones128f[:], counts128[:], start=True, stop=True)
        crow = plab.tile([1, C], FP32)
        nc.vector.tensor_copy(crow[:], cnt_ps[:])
        cntT_ps = psm.tile([16, 1], FP32, tag="ps_misc")
        nc.tensor.transpose(cntT_ps[:], crow[:], I128f[0:1, 0:1])
        counts = plab.tile([16, 1], FP32)
        nc.vector.tensor_copy(counts[:], cntT_ps[:])
        recip = plab.tile([16, 1], FP32)
        nc.vector.reciprocal(recip[:], counts[:])
        protom = plab.tile([16, 128], FP32)
        nc.vector.tensor_scalar(protom[:], protosum[:], recip[:], None, op0=ALU.mult)
        pT_ps = psm.tile([128, 128], FP32, tag="ps_misc")
        nc.tensor.transpose(pT_ps[0:128, 0:16], protom[:], I128f[0:16, 0:16])
        protosTf = plab.tile([128, C], FP32)
        nc.vector.tensor_copy(protosTf[:], pT_ps[0:128, 0:16])

        # |v| = 2*relu(v) - v  =>  dist[c,n] = 2*(sum_d relu(x-p) - xs1[n]/2) + ps1[c]
        # the -xs1/2 term lands in the same PSUM via a -0.5*ones matmul.
        ps1s = plab.tile([16, 1], FP32)
        nc.vector.tensor_reduce(ps1s[:], protom[:], axis=AX.X, op=ALU.add)

        # ---------------- distances ----------------
        dists_q = [
            ppost.tile([128, GLOC // 4, C], BF16, name=f"dists{q}", tag=f"dists{q}")
            for q in range(4)
        ]
        BLKW = 2048
        NBLK = NLOC // BLKW  # 4 blocks x 4 chunks of 512
        for blk in range(NBLK):
            dist_ps = [psdist.tile([16, 512], FP32, tag="dist", name=f"dist{blk}_{j}") for j in range(4)]
            for c in range(C):
                ad = pad.tile([128, BLKW], BF16, tag="ad")
                nc.vector.tensor_scalar(
                    ad[:], xb[:, bass.ds(BLKW * blk, BLKW)],
                    protosTf[:, c : c + 1], 0.0,
                    op0=ALU.subtract, op1=ALU.max,
                )
                for j in range(4):
                    nc.tensor.matmul(
                        dist_ps[j][:], E_b[:, c, :], ad[:, bass.ts(j, 512)],
                        start=(c == 0), stop=False, skip_group_check=True,
                    )
            for j in range(4):
                nc.tensor.matmul(
                    dist_ps[j][:], negh[:],
                    xb[:, bass.ds(BLKW * blk + 512 * j, 512)],
                    start=False, stop=True, skip_group_check=True,
                )
            for j in range(4):
                # dsb = 2*psum + ps1 (fused into the PSUM->SBUF copy), bf16
                dsb = pdsb.tile([16, 512], BF16)
                nc.scalar.activation(
                    dsb[:], dist_ps[j][:], AF.Identity, bias=ps1s[:], scale=2.0
                )
                # DMA-xbar transpose [16,512] -> [128, 4, 16] (pixel-major)
                nc.sync.dma_start_transpose(
                    dists_q[blk][:, 4 * j : 4 * j + 4, :], dsb[:]
                )

        # ---------------- post-processing (softmax CE) ----------------
        ohp = poh.tile([128, GLOC, C], BF16)  # local-half onehot
        for g in range(GLOC):
            nc.gpsimd.tensor_scalar(
                ohp[:, g, :], iotaCf[:], labT[:, g : g + 1], None, op0=ALU.is_equal
            )
        # per-quarter post (overlaps with the next distance block's matmuls)
        GB = GLOC // NBLK  # 16 tiles per quarter
        mce = ppost.tile([128, GLOC], FP32)
        for q in range(NBLK):
            g0 = q * GB
            e = ppost.tile([128, GB, C], FP32, name=f"e{q}", tag="e", bufs=2)
            z = ppost.tile([128, GB, C], FP32, name=f"z{q}", tag="z", bufs=2)
            s = ppost.tile([128, GB], FP32, name=f"s{q}", tag="s", bufs=2)
            lse = ppost.tile([128, GB], FP32, name=f"lse{q}", tag="lse", bufs=2)
            dsel = ppost.tile([128, GB], FP32, name=f"dsel{q}", tag="dsel", bufs=2)
            esel = ppost.tile([128, GB], FP32, name=f"esel{q}", tag="esel", bufs=2)
            ce = ppost.tile([128, GB], FP32, name=f"ce{q}", tag="ce", bufs=2)
            e2 = e[:].rearrange("p g c -> p (g c)")
            z2 = z[:].rearrange("p g c -> p (g c)")
            dq = dists_q[q][:].rearrange("p g c -> p (g c)")
            nc.scalar.activation(e2, dq, AF.Exp, scale=-1.0)
            nc.scalar.activation(z2, e2, AF.Exp)
            nc.vector.tensor_reduce(s[:], z[:], axis=AX.X, op=ALU.add)
            nc.scalar.activation(lse[:], s[:], AF.Ln)
            # dist at label class: sum_c dists*onehot, then exp(-x)
            oq = ohp[:, g0 : g0 + GB, :].rearrange("p g c -> p (g c)")
            nc.vector.tensor_mul(z2, dq, oq)
            nc.vector.tensor_reduce(dsel[:], z[:], axis=AX.X, op=ALU.add)
            nc.scalar.activation(esel[:], dsel[:], AF.Exp, scale=-1.0)
            nc.vector.tensor_sub(ce[:], lse[:], esel[:])
            nc.vector.tensor_mul(mce[:, g0 : g0 + GB], ce[:], mskT[:, g0 : g0 + GB])

        dbg = ppost.tile([128, 4, C], FP32)
        nc.vector.tensor_copy(dbg[:], dists_q[0][:, 0:4, :])
        nc.sync.dma_start(dbg_ap[:], dbg[:])

        outsb = ppost.tile([128, 2], FP32)
        nc.vector.tensor_reduce(outsb[:, 0:1], mce[:], axis=AX.X, op=ALU.add)
        nc.vector.tensor_reduce(outsb[:, 1:2], mskT[:], axis=AX.X, op=ALU.add)
        nc.sync.dma_start(out_ap[:], outsb[:])


_CACHE = {}


def _build():
    if "nc" in _CACHE:
        return _CACHE["nc"]
    nc = bacc.Bacc("TRN2", target_bir_lowering=False, debug=False, num_devices=8)
    xb_ap = nc.dram_tensor("xb", [128, N], BF16, kind="ExternalInput").ap()
    lab_ap = nc.dram_tensor("lab", [128, 128], I32, kind="ExternalInput").ap()
    msk_ap = nc.dram_tensor("msk", [64, 128], I32, kind="ExternalInput").ap()
    out_ap = nc.dram_tensor("acc", [128, 2], FP32, kind="ExternalOutput").ap()
    dbg_ap = nc.dram_tensor("dbg", [128, 4, C], FP32, kind="ExternalOutput").ap()
    with tile.TileContext(nc) as tc:
        build_body(tc, out_ap, dbg_ap, xb_ap, lab_ap, msk_ap)
    nc.compile()
    _CACHE["nc"] = nc
    return nc


def make_in_maps(batch_x, batch_label, batch_mask):
    import ml_dtypes

    xb_all = np.asarray(batch_x).astype(ml_dtypes.bfloat16)  # one 32->16MB cast
    in_maps = []
    for k in range(8):
        b, h = k // 2, k % 2
        loc = slice(64 * h, 64 * h + 64)
        oth = slice(64 - 64 * h, 128 - 64 * h)
        xf = np.concatenate(
            [
                xb_all[b][:, loc, :].reshape(128, NLOC),
                xb_all[b][:, oth, :].reshape(128, NLOC),
            ],
            axis=1,
        )
        lab = np.concatenate([batch_label[b][loc], batch_label[b][oth]], axis=0)
        msk = batch_mask[b, 0][loc]
        in_maps.append(
            {
                "xb": xf,
                "lab": np.ascontiguousarray(lab, dtype=np.int32),
                "msk": np.ascontiguousarray(msk, dtype=np.int32),
            }
        )
    return in_maps


def combine(outs):
    total = 0.0
    for b in range(4):
        s = float(outs[2 * b][:, 0].sum()) + float(outs[2 * b + 1][:, 0].sum())
        m = float(outs[2 * b][:, 1].sum()) + float(outs[2 * b + 1][:, 1].sum())
        total += np.float32(s) / np.float32(m)
    return np.asarray(total, dtype=np.float32)


def kernel(batch_x, batch_label, batch_mask, **kw):
    nc = _build()
    in_maps = make_in_maps(
        np.asarray(batch_x), np.asarray(batch_label), np.asarray(batch_mask)
    )
    res = run_bass_kernel_spmd(nc, in_maps, core_ids=list(range(8)), **kw)
    outs = [r["acc"] for r in res.results]
    out = combine(outs)
    if kw:
        return out, res
    return out
